# revision 5
# baseline (speedup 1.0000x reference)
"""Trainium2 Bass kernel for nn_BattleEmbeddingModule.

Computes, for battle [B, 10451] fp32:
  out = concat([player@Wp.T, status@Ws.T, pinfo@Wi.T, cards(13)@Wc.T,
                potions@Wpo.T, relics@Wr.T, monsters(5)@Wm.T], -1)  -> [B, 542]

Strategy: pure data parallel over 8 NeuronCores (4096 rows each).
Per core, for each 512-row group:
  1. DMA battle rows in natural layout (segment-aligned feature blocks)
  2. PE-transpose 128x(<=128) windows (fp32, exact) into PSUM
  3. DVE/ACT copy PSUM -> SBUF with fp32r rounding (TF32)
  4. fp32r matmuls, weights stationary, accumulate out.T chains in PSUM
  5. chain PSUM -> SBUF (packed 4 chains / 128 partitions)
  6. PE-transpose back to natural layout, compact, DMA out

All matmul operands/outputs live at partition base 0 (HW requirement seen
on this toolchain); engine copies use 32-aligned partition bases only.
"""

import sys
import types

import numpy as np

# ---------------------------------------------------------------------------
# NTFF profile hook shim: lets trace=True work when the harness requests
# profiling (BASS_TRACE=1) in a container whose antenv lacks axon_hooks.
def _install_ntff_shim():
    try:
        if "antenv.axon_hooks" in sys.modules:
            return
        import antenv

        mod = types.ModuleType("antenv.axon_hooks")
        _state = {"hook": None}
        mod.set_axon_ntff_profile_hook = lambda h: _state.__setitem__("hook", h)
        mod.get_axon_ntff_profile_hook = lambda: _state["hook"]
        sys.modules["antenv.axon_hooks"] = mod
        antenv.axon_hooks = mod
        from trn_agent_boot.trn_boot import _ntff_profile_via_ctypes

        mod.set_axon_ntff_profile_hook(
            _ntff_profile_via_ctypes("/opt/axon/libaxon_pjrt.so")
        )
    except Exception:
        pass


_install_ntff_shim()

import concourse.bacc as bacc
import concourse.mybir as mybir
import concourse.tile as tile
from concourse.bass_utils import run_bass_kernel_spmd

F32 = mybir.dt.float32
F32R = mybir.dt.float32r

# ---------------------------------------------------------------------------
# Problem geometry (hardcoded from the module definition)
N_CORES = 8
B_FULL = 32768
D_IN = 10451
D_OUT = 542
ROWS_PER_CORE = B_FULL // N_CORES  # 4096
GROUP = 512                        # rows per processing group (psum N)
SUB = 128                          # rows per PE transpose

# segments: (f_start, f_len, o_start, m)  -- seg1 packs player+status+pinfo
SEGS = (
    [(0, 103, 0, 22)]
    + [(103 + 740 * j, 740, 22 + 32 * j, 32) for j in range(13)]
    + [(9723, 43, 438, 8), (9766, 180, 446, 16)]
    + [(9946 + 101 * j, 101, 462 + 16 * j, 16) for j in range(5)]
)
N_CHAINS = len(SEGS)  # 21

# feature blocks for DMA streaming (segment-aligned)
FBLK = [0, 843, 2323, 3803, 5283, 6763, 8243, 9723, D_IN]

# windows: (chain, k_idx, n_k, f_abs, fw, wcol)
WINDOWS = []
_wcol = 0
for _c, (_f0, _fl, _o0, _m) in enumerate(SEGS):
    _n_k = (_fl + 127) // 128
    for _k in range(_n_k):
        _fa = _f0 + 128 * _k
        _fw = min(128, _f0 + _fl - _fa)
        WINDOWS.append((_c, _k, _n_k, _fa, _fw, _wcol))
        _wcol += _m
WCOLS = _wcol  # 2638


def _blk_of(f_abs):
    for b in range(len(FBLK) - 1):
        if FBLK[b] <= f_abs < FBLK[b + 1]:
            return b
    raise AssertionError(f_abs)


# chain slot -> oT tile (4 chains of <=32 partitions per 128-partition tile)
# tile t covers chains 4t..4t+3
N_OT = (N_CHAINS + 3) // 4  # 6

# detranspose targets: onat_a covers oT tiles 0..3 (padded cols 0..512),
# onat_b covers tiles 4..5 (padded cols 0..192)
# padded col of chain c = 128*(c//4) + 32*(c%4) ... within its half


def build_nc(rows_per_core):
    n_groups = rows_per_core // GROUP
    assert n_groups * GROUP == rows_per_core

    nc = bacc.Bacc(
        "TRN2", target_bir_lowering=False, debug=False, num_devices=N_CORES
    )

    battle_d = nc.dram_tensor(
        "battle", [rows_per_core, D_IN], F32, kind="ExternalInput"
    ).ap()
    wpack_d = nc.dram_tensor("wpack", [128, WCOLS], F32, kind="ExternalInput").ap()
    ident_d = nc.dram_tensor("ident", [128, 128], F32, kind="ExternalInput").ap()
    out_d = nc.dram_tensor(
        "out", [rows_per_core, D_OUT], F32, kind="ExternalOutput"
    ).ap()

    with tile.TileContext(nc) as tc:
        with (
            tc.tile_pool(name="const", bufs=1) as const_pool,
            tc.tile_pool(name="xnat", bufs=12) as xnat_pool,
            tc.tile_pool(name="xt", bufs=6) as xt_pool,
            tc.tile_pool(name="ot", bufs=12) as ot_pool,
            tc.tile_pool(name="onsb", bufs=4) as onsb_pool,
            tc.tile_pool(name="ps_stage", bufs=3, space="PSUM") as ps_stage,
            tc.tile_pool(name="ps_chain", bufs=3, space="PSUM") as ps_chain,
            tc.tile_pool(name="ps_onat", bufs=2, space="PSUM") as ps_onat,
        ):
            ident = const_pool.tile([128, 128], F32, name="ident_sb", tag="ident")
            nc.sync.dma_start(out=ident, in_=ident_d)
            wpack = const_pool.tile(
                [128, WCOLS], F32R, name="wpack_sb", tag="wpack"
            )
            nc.gpsimd.dma_start(out=wpack, in_=wpack_d)

            copy_flip = 0
            chain_ps = {}

            for g in range(n_groups):
                r0 = g * GROUP
                # --- DMA in: 4 row-subtiles x 8 feature blocks
                xnat = [[None] * (len(FBLK) - 1) for _ in range(4)]
                for b in range(len(FBLK) - 1):
                    fl = FBLK[b + 1] - FBLK[b]
                    for r in range(4):
                        t = xnat_pool.tile([SUB, 1480], F32, name=f"xn_{g}_{r}_{b}", tag="xn")
                        nc.sync.dma_start(
                            out=t[:, :fl],
                            in_=battle_d[
                                r0 + SUB * r : r0 + SUB * (r + 1),
                                FBLK[b] : FBLK[b + 1],
                            ],
                        )
                        xnat[r][b] = t

                # --- oT output-staging tiles (4 chains each)
                ots = [
                    ot_pool.tile([128, GROUP], F32, name=f"ot_{g}_{t}", tag="ot")
                    for t in range(N_OT)
                ]

                # --- window sweep: transpose -> round-copy -> matmul chains
                for (c, k, n_k, fa, fw, wcol) in WINDOWS:
                    b = _blk_of(fa)
                    loc = fa - FBLK[b]
                    m = SEGS[c][3]

                    stg = ps_stage.tile([128, GROUP], F32, name=f"stg_{g}_{c}_{k}", tag="stg")
                    for r in range(4):
                        nc.tensor.transpose(
                            stg[:fw, SUB * r : SUB * (r + 1)],
                            xnat[r][b][:, loc : loc + fw],
                            ident,
                        )
                    xt = xt_pool.tile([128, GROUP], F32R, name=f"xt_{g}_{c}_{k}", tag="xt")
                    if copy_flip % 2 == 0:
                        nc.vector.tensor_copy(xt[:fw, :], stg[:fw, :])
                    else:
                        nc.scalar.copy(xt[:fw, :], stg[:fw, :])
                    copy_flip += 1

                    if k == 0:
                        chain_ps[c] = ps_chain.tile(
                            [32, GROUP], F32, name=f"ch_{g}_{c}", tag="ch"
                        )
                    ch = chain_ps[c]
                    nc.tensor.matmul(
                        ch[:m, :],
                        wpack[:fw, wcol : wcol + m],
                        xt[:fw, :],
                        start=(k == 0),
                        stop=(k == n_k - 1),
                    )
                    if k == n_k - 1:
                        # pack chain into its oT tile at 32-aligned offset
                        t_i, j = c // 4, c % 4
                        nc.vector.tensor_copy(
                            ots[t_i][32 * j : 32 * j + m, :], ch[:m, :]
                        )

                # --- detranspose to natural layout + compact + DMA out
                for r in range(4):
                    onat_a = ps_onat.tile([128, 512], F32, name=f"ona_{g}_{r}", tag="onat")
                    onat_b = ps_onat.tile([128, 512], F32, name=f"onb_{g}_{r}", tag="onat")
                    for t_i in range(N_OT):
                        valid = min(128, 32 * (N_CHAINS - 4 * t_i))
                        src = ots[t_i][:valid, SUB * r : SUB * (r + 1)]
                        if t_i < 4:
                            dst = onat_a[:, 128 * t_i : 128 * t_i + valid]
                        else:
                            off = 128 * (t_i - 4)
                            dst = onat_b[:, off : off + valid]
                        nc.tensor.transpose(dst, src, ident[:valid, :valid])

                    onat = onsb_pool.tile([SUB, D_OUT], F32, name=f"on_{g}_{r}", tag="onsb")
                    # compact copies: padded slot layout -> true output layout
                    nc.scalar.copy(onat[:, 0:22], onat_a[:, 0:22])
                    nc.scalar.copy(onat[:, 22:438], onat_a[:, 32:448])
                    nc.scalar.copy(onat[:, 438:446], onat_a[:, 448:456])
                    nc.scalar.copy(onat[:, 446:462], onat_a[:, 480:496])
                    nc.vector.tensor_copy(
                        onat[:, 462:542].rearrange("p (a b) -> p a b", a=5),
                        onat_b[:, 0:160].rearrange("p (a b) -> p a b", a=5)[
                            :, :, 0:16
                        ],
                    )
                    nc.sync.dma_start(
                        out=out_d[r0 + SUB * r : r0 + SUB * (r + 1), :], in_=onat
                    )

    nc.compile()
    return nc


_NC_CACHE = {}


def _get_nc(rows_per_core):
    if rows_per_core not in _NC_CACHE:
        _NC_CACHE[rows_per_core] = build_nc(rows_per_core)
    return _NC_CACHE[rows_per_core]


def pack_weights(W_player, W_status, W_pinfo, W_card, W_potions, W_relics,
                 W_monster):
    wpack = np.zeros((128, WCOLS), dtype=np.float32)
    wt = {}
    bd = np.zeros((103, 22), dtype=np.float32)
    bd[0:9, 0:4] = np.asarray(W_player, np.float32).T
    bd[9:95, 4:20] = np.asarray(W_status, np.float32).T
    bd[95:103, 20:22] = np.asarray(W_pinfo, np.float32).T
    wt[0] = bd
    for j in range(13):
        wt[1 + j] = np.asarray(W_card, np.float32).T
    wt[14] = np.asarray(W_potions, np.float32).T
    wt[15] = np.asarray(W_relics, np.float32).T
    for j in range(5):
        wt[16 + j] = np.asarray(W_monster, np.float32).T
    for (c, k, n_k, fa, fw, wcol) in WINDOWS:
        m = SEGS[c][3]
        f_rel = fa - SEGS[c][0]
        wpack[:fw, wcol : wcol + m] = wt[c][f_rel : f_rel + fw, :]
    return wpack


def run_sharded(inputs, rows_per_core=ROWS_PER_CORE, trace=False,
                trace_kwargs=None):
    """Shard along batch, run the SPMD kernel, gather. Returns (out, results)."""
    battle = np.ascontiguousarray(np.asarray(inputs["battle"], np.float32))
    n_rows = battle.shape[0]
    assert n_rows == rows_per_core * N_CORES
    wpack = pack_weights(
        inputs["W_player"], inputs["W_status"], inputs["W_pinfo"],
        inputs["W_card"], inputs["W_potions"], inputs["W_relics"],
        inputs["W_monster"],
    )
    ident = np.eye(128, dtype=np.float32)
    nc = _get_nc(rows_per_core)
    in_maps = [
        {
            "battle": battle[c * rows_per_core : (c + 1) * rows_per_core],
            "wpack": wpack,
            "ident": ident,
        }
        for c in range(N_CORES)
    ]
    res = run_bass_kernel_spmd(
        nc, in_maps, list(range(N_CORES)), trace=trace,
        **(trace_kwargs or {}),
    )
    out = np.concatenate([res.results[c]["out"] for c in range(N_CORES)], axis=0)
    return out, res


def kernel(**inputs) -> np.ndarray:
    out, _ = run_sharded(inputs)
    return out


# revision 6
# speedup vs baseline: 1.0573x; 1.0573x over previous
"""Trainium2 Bass kernel for nn_BattleEmbeddingModule.

Computes, for battle [B, 10451] fp32:
  out = concat([player@Wp.T, status@Ws.T, pinfo@Wi.T, cards(13)@Wc.T,
                potions@Wpo.T, relics@Wr.T, monsters(5)@Wm.T], -1)  -> [B, 542]

Strategy: pure data parallel over 8 NeuronCores (4096 rows each).
Per core, for each 512-row group:
  1. DMA battle rows in natural layout (segment-aligned feature blocks)
  2. PE-transpose 128x(<=128) windows (fp32, exact) into PSUM
  3. DVE/ACT copy PSUM -> SBUF with fp32r rounding (TF32)
  4. fp32r matmuls, weights stationary, accumulate out.T chains in PSUM
  5. chain PSUM -> SBUF (packed 4 chains / 128 partitions)
  6. PE-transpose back to natural layout, compact, DMA out

All matmul operands/outputs live at partition base 0 (HW requirement seen
on this toolchain); engine copies use 32-aligned partition bases only.
"""

import sys
import types

import numpy as np

# ---------------------------------------------------------------------------
# NTFF profile hook shim: lets trace=True work when the harness requests
# profiling (BASS_TRACE=1) in a container whose antenv lacks axon_hooks.
def _install_ntff_shim():
    try:
        if "antenv.axon_hooks" in sys.modules:
            return
        import antenv

        mod = types.ModuleType("antenv.axon_hooks")
        _state = {"hook": None}
        mod.set_axon_ntff_profile_hook = lambda h: _state.__setitem__("hook", h)
        mod.get_axon_ntff_profile_hook = lambda: _state["hook"]
        sys.modules["antenv.axon_hooks"] = mod
        antenv.axon_hooks = mod
        from trn_agent_boot.trn_boot import _ntff_profile_via_ctypes

        mod.set_axon_ntff_profile_hook(
            _ntff_profile_via_ctypes("/opt/axon/libaxon_pjrt.so")
        )
    except Exception:
        pass


_install_ntff_shim()

import concourse.bacc as bacc
import concourse.mybir as mybir
import concourse.tile as tile
from concourse.bass_utils import run_bass_kernel_spmd

F32 = mybir.dt.float32
F32R = mybir.dt.float32r

# ---------------------------------------------------------------------------
# Problem geometry (hardcoded from the module definition)
N_CORES = 8
B_FULL = 32768
D_IN = 10451
D_OUT = 542
ROWS_PER_CORE = B_FULL // N_CORES  # 4096
GROUP = 512                        # rows per processing group (psum N)
SUB = 128                          # rows per PE transpose

# segments: (f_start, f_len, o_start, m)  -- seg1 packs player+status+pinfo
SEGS = (
    [(0, 103, 0, 22)]
    + [(103 + 740 * j, 740, 22 + 32 * j, 32) for j in range(13)]
    + [(9723, 43, 438, 8), (9766, 180, 446, 16)]
    + [(9946 + 101 * j, 101, 462 + 16 * j, 16) for j in range(5)]
)
N_CHAINS = len(SEGS)  # 21

# feature blocks for DMA streaming (segment-aligned)
FBLK = [0, 843, 2323, 3803, 5283, 6763, 8243, 9723, D_IN]

# windows: (chain, k_idx, n_k, f_abs, fw, wcol)
WINDOWS = []
_wcol = 0
for _c, (_f0, _fl, _o0, _m) in enumerate(SEGS):
    _n_k = (_fl + 127) // 128
    for _k in range(_n_k):
        _fa = _f0 + 128 * _k
        _fw = min(128, _f0 + _fl - _fa)
        WINDOWS.append((_c, _k, _n_k, _fa, _fw, _wcol))
        _wcol += _m
WCOLS = _wcol  # 2638


def _blk_of(f_abs):
    for b in range(len(FBLK) - 1):
        if FBLK[b] <= f_abs < FBLK[b + 1]:
            return b
    raise AssertionError(f_abs)


# chain slot -> oT tile (4 chains of <=32 partitions per 128-partition tile)
# tile t covers chains 4t..4t+3
N_OT = (N_CHAINS + 3) // 4  # 6

# detranspose targets: onat_a covers oT tiles 0..3 (padded cols 0..512),
# onat_b covers tiles 4..5 (padded cols 0..192)
# padded col of chain c = 128*(c//4) + 32*(c%4) ... within its half


def build_nc(rows_per_core):
    n_groups = rows_per_core // GROUP
    assert n_groups * GROUP == rows_per_core

    nc = bacc.Bacc(
        "TRN2", target_bir_lowering=False, debug=False, num_devices=N_CORES
    )

    battle_d = nc.dram_tensor(
        "battle", [rows_per_core, D_IN], F32, kind="ExternalInput"
    ).ap()
    wpack_d = nc.dram_tensor("wpack", [128, WCOLS], F32, kind="ExternalInput").ap()
    ident_d = nc.dram_tensor("ident", [128, 128], F32, kind="ExternalInput").ap()
    out_d = nc.dram_tensor(
        "out", [rows_per_core, D_OUT], F32, kind="ExternalOutput"
    ).ap()

    with tile.TileContext(nc) as tc:
        with (
            tc.tile_pool(name="const", bufs=1) as const_pool,
            tc.tile_pool(name="xnat", bufs=12) as xnat_pool,
            tc.tile_pool(name="xt", bufs=6) as xt_pool,
            tc.tile_pool(name="ot", bufs=12) as ot_pool,
            tc.tile_pool(name="onsb", bufs=4) as onsb_pool,
            tc.tile_pool(name="ps_stage", bufs=4, space="PSUM") as ps_stage,
            tc.tile_pool(name="ps_chain", bufs=2, space="PSUM") as ps_chain,
            tc.tile_pool(name="ps_onat", bufs=2, space="PSUM") as ps_onat,
        ):
            ident = const_pool.tile([128, 128], F32, name="ident_sb", tag="ident")
            nc.sync.dma_start(out=ident, in_=ident_d)
            identr = const_pool.tile(
                [128, 128], F32R, name="identr_sb", tag="identr"
            )
            nc.gpsimd.dma_start(out=identr, in_=ident_d)
            wpack = const_pool.tile(
                [128, WCOLS], F32R, name="wpack_sb", tag="wpack"
            )
            nc.gpsimd.dma_start(out=wpack, in_=wpack_d)

            copy_flip = 0
            chain_ps = {}

            for g in range(n_groups):
                r0 = g * GROUP
                # --- DMA in: 4 row-subtiles x 8 feature blocks
                xnat = [[None] * (len(FBLK) - 1) for _ in range(4)]
                for b in range(len(FBLK) - 1):
                    fl = FBLK[b + 1] - FBLK[b]
                    for r in range(4):
                        t = xnat_pool.tile([SUB, 1480], F32R, name=f"xn_{g}_{r}_{b}", tag="xn")
                        nc.gpsimd.dma_start(
                            out=t[:, :fl],
                            in_=battle_d[
                                r0 + SUB * r : r0 + SUB * (r + 1),
                                FBLK[b] : FBLK[b + 1],
                            ],
                        )
                        xnat[r][b] = t

                # --- oT output-staging tiles (4 chains each)
                ots = [
                    ot_pool.tile([128, GROUP], F32, name=f"ot_{g}_{t}", tag="ot")
                    for t in range(N_OT)
                ]

                # --- window sweep: transpose -> round-copy -> matmul chains
                for (c, k, n_k, fa, fw, wcol) in WINDOWS:
                    b = _blk_of(fa)
                    loc = fa - FBLK[b]
                    m = SEGS[c][3]

                    stg = ps_stage.tile([128, GROUP], F32R, name=f"stg_{g}_{c}_{k}", tag="stg")
                    for r in range(4):
                        nc.tensor.transpose(
                            stg[:fw, SUB * r : SUB * (r + 1)],
                            xnat[r][b][:, loc : loc + fw],
                            identr,
                        )
                    xt = xt_pool.tile([128, GROUP], F32R, name=f"xt_{g}_{c}_{k}", tag="xt")
                    if copy_flip % 2 == 0:
                        nc.vector.tensor_copy(xt[:fw, :], stg[:fw, :])
                    else:
                        nc.scalar.copy(xt[:fw, :], stg[:fw, :])
                    copy_flip += 1

                    if k == 0:
                        chain_ps[c] = ps_chain.tile(
                            [32, GROUP], F32, name=f"ch_{g}_{c}", tag="ch"
                        )
                    ch = chain_ps[c]
                    nc.tensor.matmul(
                        ch[:m, :],
                        wpack[:fw, wcol : wcol + m],
                        xt[:fw, :],
                        start=(k == 0),
                        stop=(k == n_k - 1),
                    )
                    if k == n_k - 1:
                        # pack chain into its oT tile at 32-aligned offset
                        t_i, j = c // 4, c % 4
                        nc.vector.tensor_copy(
                            ots[t_i][32 * j : 32 * j + m, :], ch[:m, :]
                        )

                # --- detranspose to natural layout + compact + DMA out
                for r in range(4):
                    onat_a = ps_onat.tile([128, 512], F32, name=f"ona_{g}_{r}", tag="onat")
                    onat_b = ps_onat.tile([128, 512], F32, name=f"onb_{g}_{r}", tag="onat")
                    for t_i in range(N_OT):
                        valid = min(128, 32 * (N_CHAINS - 4 * t_i))
                        src = ots[t_i][:valid, SUB * r : SUB * (r + 1)]
                        if t_i < 4:
                            dst = onat_a[:, 128 * t_i : 128 * t_i + valid]
                        else:
                            off = 128 * (t_i - 4)
                            dst = onat_b[:, off : off + valid]
                        nc.tensor.transpose(dst, src, ident[:valid, :valid])

                    onat = onsb_pool.tile([SUB, D_OUT], F32, name=f"on_{g}_{r}", tag="onsb")
                    # compact copies: padded slot layout -> true output layout
                    nc.scalar.copy(onat[:, 0:22], onat_a[:, 0:22])
                    nc.scalar.copy(onat[:, 22:438], onat_a[:, 32:448])
                    nc.scalar.copy(onat[:, 438:446], onat_a[:, 448:456])
                    nc.scalar.copy(onat[:, 446:462], onat_a[:, 480:496])
                    nc.vector.tensor_copy(
                        onat[:, 462:542].rearrange("p (a b) -> p a b", a=5),
                        onat_b[:, 0:160].rearrange("p (a b) -> p a b", a=5)[
                            :, :, 0:16
                        ],
                    )
                    nc.sync.dma_start(
                        out=out_d[r0 + SUB * r : r0 + SUB * (r + 1), :], in_=onat
                    )

    nc.compile()
    return nc


_NC_CACHE = {}


def _get_nc(rows_per_core):
    if rows_per_core not in _NC_CACHE:
        _NC_CACHE[rows_per_core] = build_nc(rows_per_core)
    return _NC_CACHE[rows_per_core]


def pack_weights(W_player, W_status, W_pinfo, W_card, W_potions, W_relics,
                 W_monster):
    wpack = np.zeros((128, WCOLS), dtype=np.float32)
    wt = {}
    bd = np.zeros((103, 22), dtype=np.float32)
    bd[0:9, 0:4] = np.asarray(W_player, np.float32).T
    bd[9:95, 4:20] = np.asarray(W_status, np.float32).T
    bd[95:103, 20:22] = np.asarray(W_pinfo, np.float32).T
    wt[0] = bd
    for j in range(13):
        wt[1 + j] = np.asarray(W_card, np.float32).T
    wt[14] = np.asarray(W_potions, np.float32).T
    wt[15] = np.asarray(W_relics, np.float32).T
    for j in range(5):
        wt[16 + j] = np.asarray(W_monster, np.float32).T
    for (c, k, n_k, fa, fw, wcol) in WINDOWS:
        m = SEGS[c][3]
        f_rel = fa - SEGS[c][0]
        wpack[:fw, wcol : wcol + m] = wt[c][f_rel : f_rel + fw, :]
    return wpack


def run_sharded(inputs, rows_per_core=ROWS_PER_CORE, trace=False,
                trace_kwargs=None):
    """Shard along batch, run the SPMD kernel, gather. Returns (out, results)."""
    battle = np.ascontiguousarray(np.asarray(inputs["battle"], np.float32))
    n_rows = battle.shape[0]
    assert n_rows == rows_per_core * N_CORES
    wpack = pack_weights(
        inputs["W_player"], inputs["W_status"], inputs["W_pinfo"],
        inputs["W_card"], inputs["W_potions"], inputs["W_relics"],
        inputs["W_monster"],
    )
    ident = np.eye(128, dtype=np.float32)
    nc = _get_nc(rows_per_core)
    in_maps = [
        {
            "battle": battle[c * rows_per_core : (c + 1) * rows_per_core],
            "wpack": wpack,
            "ident": ident,
        }
        for c in range(N_CORES)
    ]
    res = run_bass_kernel_spmd(
        nc, in_maps, list(range(N_CORES)), trace=trace,
        **(trace_kwargs or {}),
    )
    out = np.concatenate([res.results[c]["out"] for c in range(N_CORES)], axis=0)
    return out, res


def kernel(**inputs) -> np.ndarray:
    out, _ = run_sharded(inputs)
    return out


# revision 7
# speedup vs baseline: 1.0844x; 1.0256x over previous
"""Trainium2 Bass kernel for nn_BattleEmbeddingModule.

Computes, for battle [B, 10451] fp32:
  out = concat([player@Wp.T, status@Ws.T, pinfo@Wi.T, cards(13)@Wc.T,
                potions@Wpo.T, relics@Wr.T, monsters(5)@Wm.T], -1)  -> [B, 542]

Strategy: pure data parallel over 8 NeuronCores (4096 rows each).
Per core, for each 512-row group:
  1. DMA battle rows in natural layout (segment-aligned feature blocks)
  2. PE-transpose 128x(<=128) windows (fp32, exact) into PSUM
  3. DVE/ACT copy PSUM -> SBUF with fp32r rounding (TF32)
  4. fp32r matmuls, weights stationary, accumulate out.T chains in PSUM
  5. chain PSUM -> SBUF (packed 4 chains / 128 partitions)
  6. PE-transpose back to natural layout, compact, DMA out

All matmul operands/outputs live at partition base 0 (HW requirement seen
on this toolchain); engine copies use 32-aligned partition bases only.
"""

import sys
import types

import numpy as np

# ---------------------------------------------------------------------------
# NTFF profile hook shim: lets trace=True work when the harness requests
# profiling (BASS_TRACE=1) in a container whose antenv lacks axon_hooks.
def _install_ntff_shim():
    try:
        if "antenv.axon_hooks" in sys.modules:
            return
        import antenv

        mod = types.ModuleType("antenv.axon_hooks")
        _state = {"hook": None}
        mod.set_axon_ntff_profile_hook = lambda h: _state.__setitem__("hook", h)
        mod.get_axon_ntff_profile_hook = lambda: _state["hook"]
        sys.modules["antenv.axon_hooks"] = mod
        antenv.axon_hooks = mod
        from trn_agent_boot.trn_boot import _ntff_profile_via_ctypes

        mod.set_axon_ntff_profile_hook(
            _ntff_profile_via_ctypes("/opt/axon/libaxon_pjrt.so")
        )
    except Exception:
        pass


_install_ntff_shim()

import concourse.bacc as bacc
import concourse.mybir as mybir
import concourse.tile as tile
from concourse.bass_utils import run_bass_kernel_spmd

F32 = mybir.dt.float32
F32R = mybir.dt.float32r

# ---------------------------------------------------------------------------
# Problem geometry (hardcoded from the module definition)
N_CORES = 8
B_FULL = 32768
D_IN = 10451
D_OUT = 542
ROWS_PER_CORE = B_FULL // N_CORES  # 4096
GROUP = 512                        # rows per processing group (psum N)
SUB = 128                          # rows per PE transpose

# segments: (f_start, f_len, o_start, m)  -- seg1 packs player+status+pinfo
SEGS = (
    [(0, 103, 0, 22)]
    + [(103 + 740 * j, 740, 22 + 32 * j, 32) for j in range(13)]
    + [(9723, 43, 438, 8), (9766, 180, 446, 16)]
    + [(9946 + 101 * j, 101, 462 + 16 * j, 16) for j in range(5)]
)
N_CHAINS = len(SEGS)  # 21

# feature blocks for DMA streaming (segment-aligned)
FBLK = [0, 843, 2323, 3803, 5283, 6763, 8243, 9723, D_IN]

# windows: (chain, k_idx, n_k, f_abs, fw, wcol)
WINDOWS = []
_wcol = 0
for _c, (_f0, _fl, _o0, _m) in enumerate(SEGS):
    _n_k = (_fl + 127) // 128
    for _k in range(_n_k):
        _fa = _f0 + 128 * _k
        _fw = min(128, _f0 + _fl - _fa)
        WINDOWS.append((_c, _k, _n_k, _fa, _fw, _wcol))
        _wcol += _m
WCOLS = _wcol  # 2638


def _blk_of(f_abs):
    for b in range(len(FBLK) - 1):
        if FBLK[b] <= f_abs < FBLK[b + 1]:
            return b
    raise AssertionError(f_abs)


# chain slot -> oT tile (4 chains of <=32 partitions per 128-partition tile)
# tile t covers chains 4t..4t+3
N_OT = (N_CHAINS + 3) // 4  # 6

# detranspose targets: onat_a covers oT tiles 0..3 (padded cols 0..512),
# onat_b covers tiles 4..5 (padded cols 0..192)
# padded col of chain c = 128*(c//4) + 32*(c%4) ... within its half


def build_nc(rows_per_core):
    n_groups = rows_per_core // GROUP
    assert n_groups * GROUP == rows_per_core

    nc = bacc.Bacc(
        "TRN2", target_bir_lowering=False, debug=False, num_devices=N_CORES
    )

    battle_d = nc.dram_tensor(
        "battle", [rows_per_core, D_IN], F32, kind="ExternalInput"
    ).ap()
    wpack_d = nc.dram_tensor("wpack", [128, WCOLS], F32, kind="ExternalInput").ap()
    ident_d = nc.dram_tensor("ident", [128, 128], F32, kind="ExternalInput").ap()
    out_d = nc.dram_tensor(
        "out", [rows_per_core, D_OUT], F32, kind="ExternalOutput"
    ).ap()

    with tile.TileContext(nc) as tc:
        with (
            tc.tile_pool(name="const", bufs=1) as const_pool,
            tc.tile_pool(name="xnat", bufs=12) as xnat_pool,
            tc.tile_pool(name="xt", bufs=8) as xt_pool,
            tc.tile_pool(name="ot", bufs=12) as ot_pool,
            tc.tile_pool(name="onsb", bufs=4) as onsb_pool,
            tc.tile_pool(name="ps_stage", bufs=4, space="PSUM") as ps_stage,
            tc.tile_pool(name="ps_chain", bufs=2, space="PSUM") as ps_chain,
            tc.tile_pool(name="ps_onat", bufs=2, space="PSUM") as ps_onat,
        ):
            ident = const_pool.tile([128, 128], F32, name="ident_sb", tag="ident")
            nc.sync.dma_start(out=ident, in_=ident_d)
            identr = const_pool.tile(
                [128, 128], F32R, name="identr_sb", tag="identr"
            )
            nc.gpsimd.dma_start(out=identr, in_=ident_d)
            wpack = const_pool.tile(
                [128, WCOLS], F32R, name="wpack_sb", tag="wpack"
            )
            nc.gpsimd.dma_start(out=wpack, in_=wpack_d)

            copy_flip = 0
            chain_ps = {}

            for g in range(n_groups):
                r0 = g * GROUP
                # --- DMA in: 4 row-subtiles x 8 feature blocks
                xnat = [[None] * (len(FBLK) - 1) for _ in range(4)]
                for b in range(len(FBLK) - 1):
                    fl = FBLK[b + 1] - FBLK[b]
                    for r in range(4):
                        t = xnat_pool.tile([SUB, 1480], F32R, name=f"xn_{g}_{r}_{b}", tag="xn")
                        nc.gpsimd.dma_start(
                            out=t[:, :fl],
                            in_=battle_d[
                                r0 + SUB * r : r0 + SUB * (r + 1),
                                FBLK[b] : FBLK[b + 1],
                            ],
                        )
                        xnat[r][b] = t

                # --- oT output-staging tiles (4 chains each)
                ots = [
                    ot_pool.tile([128, GROUP], F32, name=f"ot_{g}_{t}", tag="ot")
                    for t in range(N_OT)
                ]

                # --- window sweep: transpose -> round-copy -> matmul chains
                for (c, k, n_k, fa, fw, wcol) in WINDOWS:
                    b = _blk_of(fa)
                    loc = fa - FBLK[b]
                    m = SEGS[c][3]

                    stg = ps_stage.tile([128, GROUP], F32R, name=f"stg_{g}_{c}_{k}", tag="stg")
                    for r in range(4):
                        nc.tensor.transpose(
                            stg[:fw, SUB * r : SUB * (r + 1)],
                            xnat[r][b][:, loc : loc + fw],
                            identr,
                        )
                    xt = xt_pool.tile([128, GROUP], F32R, name=f"xt_{g}_{c}_{k}", tag="xt")
                    # split halves across DVE+ACT: halves the copy latency and
                    # lets each half start as soon as its 2 transposes land
                    h = GROUP // 2
                    if copy_flip % 2 == 0:
                        nc.vector.tensor_copy(xt[:fw, :h], stg[:fw, :h])
                        nc.scalar.copy(xt[:fw, h:], stg[:fw, h:])
                    else:
                        nc.scalar.copy(xt[:fw, :h], stg[:fw, :h])
                        nc.vector.tensor_copy(xt[:fw, h:], stg[:fw, h:])
                    copy_flip += 1

                    if k == 0:
                        chain_ps[c] = ps_chain.tile(
                            [32, GROUP], F32, name=f"ch_{g}_{c}", tag="ch"
                        )
                    ch = chain_ps[c]
                    nc.tensor.matmul(
                        ch[:m, :],
                        wpack[:fw, wcol : wcol + m],
                        xt[:fw, :],
                        start=(k == 0),
                        stop=(k == n_k - 1),
                    )
                    if k == n_k - 1:
                        # pack chain into its oT tile at 32-aligned offset
                        t_i, j = c // 4, c % 4
                        nc.vector.tensor_copy(
                            ots[t_i][32 * j : 32 * j + m, :], ch[:m, :]
                        )

                # --- detranspose to natural layout + compact + DMA out
                for r in range(4):
                    onat_a = ps_onat.tile([128, 512], F32, name=f"ona_{g}_{r}", tag="onat")
                    onat_b = ps_onat.tile([128, 512], F32, name=f"onb_{g}_{r}", tag="onat")
                    for t_i in range(N_OT):
                        valid = min(128, 32 * (N_CHAINS - 4 * t_i))
                        src = ots[t_i][:valid, SUB * r : SUB * (r + 1)]
                        if t_i < 4:
                            dst = onat_a[:, 128 * t_i : 128 * t_i + valid]
                        else:
                            off = 128 * (t_i - 4)
                            dst = onat_b[:, off : off + valid]
                        nc.tensor.transpose(dst, src, ident[:valid, :valid])

                    onat = onsb_pool.tile([SUB, D_OUT], F32, name=f"on_{g}_{r}", tag="onsb")
                    # compact copies: padded slot layout -> true output layout
                    nc.scalar.copy(onat[:, 0:22], onat_a[:, 0:22])
                    nc.scalar.copy(onat[:, 22:438], onat_a[:, 32:448])
                    nc.scalar.copy(onat[:, 438:446], onat_a[:, 448:456])
                    nc.scalar.copy(onat[:, 446:462], onat_a[:, 480:496])
                    nc.vector.tensor_copy(
                        onat[:, 462:542].rearrange("p (a b) -> p a b", a=5),
                        onat_b[:, 0:160].rearrange("p (a b) -> p a b", a=5)[
                            :, :, 0:16
                        ],
                    )
                    nc.sync.dma_start(
                        out=out_d[r0 + SUB * r : r0 + SUB * (r + 1), :], in_=onat
                    )

    nc.compile()
    return nc


_NC_CACHE = {}


def _get_nc(rows_per_core):
    if rows_per_core not in _NC_CACHE:
        _NC_CACHE[rows_per_core] = build_nc(rows_per_core)
    return _NC_CACHE[rows_per_core]


def pack_weights(W_player, W_status, W_pinfo, W_card, W_potions, W_relics,
                 W_monster):
    wpack = np.zeros((128, WCOLS), dtype=np.float32)
    wt = {}
    bd = np.zeros((103, 22), dtype=np.float32)
    bd[0:9, 0:4] = np.asarray(W_player, np.float32).T
    bd[9:95, 4:20] = np.asarray(W_status, np.float32).T
    bd[95:103, 20:22] = np.asarray(W_pinfo, np.float32).T
    wt[0] = bd
    for j in range(13):
        wt[1 + j] = np.asarray(W_card, np.float32).T
    wt[14] = np.asarray(W_potions, np.float32).T
    wt[15] = np.asarray(W_relics, np.float32).T
    for j in range(5):
        wt[16 + j] = np.asarray(W_monster, np.float32).T
    for (c, k, n_k, fa, fw, wcol) in WINDOWS:
        m = SEGS[c][3]
        f_rel = fa - SEGS[c][0]
        wpack[:fw, wcol : wcol + m] = wt[c][f_rel : f_rel + fw, :]
    return wpack


def run_sharded(inputs, rows_per_core=ROWS_PER_CORE, trace=False,
                trace_kwargs=None):
    """Shard along batch, run the SPMD kernel, gather. Returns (out, results)."""
    battle = np.ascontiguousarray(np.asarray(inputs["battle"], np.float32))
    n_rows = battle.shape[0]
    assert n_rows == rows_per_core * N_CORES
    wpack = pack_weights(
        inputs["W_player"], inputs["W_status"], inputs["W_pinfo"],
        inputs["W_card"], inputs["W_potions"], inputs["W_relics"],
        inputs["W_monster"],
    )
    ident = np.eye(128, dtype=np.float32)
    nc = _get_nc(rows_per_core)
    in_maps = [
        {
            "battle": battle[c * rows_per_core : (c + 1) * rows_per_core],
            "wpack": wpack,
            "ident": ident,
        }
        for c in range(N_CORES)
    ]
    res = run_bass_kernel_spmd(
        nc, in_maps, list(range(N_CORES)), trace=trace,
        **(trace_kwargs or {}),
    )
    out = np.concatenate([res.results[c]["out"] for c in range(N_CORES)], axis=0)
    return out, res


def kernel(**inputs) -> np.ndarray:
    out, _ = run_sharded(inputs)
    return out
